# revision 38
# baseline (speedup 1.0000x reference)
# Trainium2 Bass kernel for nn_Lowrank_Spattention (sparse_attention).
#
# Reference math (per batch b, n=8192 tokens, f=256 features, h=4 heads,
# r=64 latent ranks, d=64 head dim):
#   q    = z @ Wq + bq                    (n, h*d)
#   attn = einsum(q, K)/sqrt(d)           (n, h*r)   == z @ M + ab
#            where M[:, h*r+j] = (Wq_h @ K_h^T)/8,  ab = bq @ K^T/8
#   xv   = x @ Wv + bv                    (n, h*d)
#   pooled = softmax_r(attn)^T-pool of xv (r, h*d)
#   v    = softmax_n(attn) @ pooled       (n, h*d)
#   out  = sig(alpha)*xv + sig(beta)*v
#
# Kernel strategy (one NeuronCore per batch element, 8 cores, no
# collectives; inputs packed into two tensors, zx = [z;x] and pk =
# byte-packed params, to minimize per-launch buffer marshaling):
#
#   Pass A is DMA-bound (16 MB of z+x loads on the serial DMA bus), so
#   ALL PE work beyond attn runs in the DMA shadow.  Per 128-row chunk:
#     zt   z^T (fp8, transient; fp8 transpose writes PSUM at element
#          step 2, copied compact to SBUF)
#     attn = zt^T @ mq as ONE fp8 DoubleRow matmul (k-halves ride the
#          [K,2,N] APs; both operands use the same (p,kt) mapping)
#     E'   = exp(attn - ln16) (bf16; the /16 keeps rowsums in fp8e4
#          range, PS is rescaled in finalize to compensate)
#     et   E'^T (bf16, resident)      for pass B's  E' @ PS_bd
#     xt   x^T (bf16, resident)       for pass B's  x @ sig(a)Wv
#          (transposed from f32r, the PSUM->SBUF copy downcasts)
#     x_res x (fp8) + aux cols [1 | rowsums' | 1] (fp8, resident)
#     G += Eh^T @ [x|aux] as fp8 DoubleRow matmuls over chunk PAIRS,
#          accumulated in PSUM over all 8192 rows; Eh = E'/rowsum'.
#   Finalize (tiny): pooled = G[:, :256] @ Wv + esum*bv;
#     PS = 16 * sig(beta) * pooled / colsum, block-diagonal (bf16).
#   Pass B is a pure matmul stream + store (PE-light, store-DMA-bound):
#     out = xt^T @ (sig(alpha)Wv) + et^T @ PS_bd (+ bias).
#
# The whole v-path (E, G, pooled) is fp8/bf16: its output contribution
# is scaled by sig(beta)=0.01 and pooled averages 8192 rows, damping
# its relative error ~1e2-1e4x.  The xv-path runs bf16 into f32 PSUM
# (~1e-3 rel err on out; tolerance is 2e-2).  Measured sim rel err
# 2.5e-3, CoreSim model time ~95 us/core vs a ~85 us DMA-floor
# (24 MB of HBM traffic at ~345 GB/s + startup/finalize/tail).
#
# Engine balance per 4-chunk quad in pass A (model): DMA 3.16 us,
# PE ~1.8, DVE ~2.7 (zt/et copies, rowsum reduce, Eh mult alt.),
# Act ~2.5 (exp, xt copies, et alt.), Pool ~2.2 (casts, Eh alt.).

import math
import os

import numpy as np

import concourse.bass as bass
import concourse.mybir as mybir
import concourse.tile as tile
from concourse import bacc

B, N, DIM = 8, 8192, 256
HEAD, RANK, HDIM = 4, 64, 64
NCORES = 8
CHUNK = 128                 # rows per compute chunk
NCHUNK = N // CHUNK         # 64
XW = DIM + 6                # x_res row width: 256 x cols + [1|rs0..3|1]

F32 = mybir.dt.float32
F32R = mybir.dt.float32r
BF16 = mybir.dt.bfloat16
F8 = mybir.dt.float8e4
Exp = mybir.ActivationFunctionType.Exp
DR = mybir.MatmulPerfMode.DoubleRow


def build_body(tc, outs, ins):
    """Emit the per-core program.  outs/ins are dicts of bass.APs."""
    nc = tc.nc
    super_ = 8                  # chunks per staged DMA
    nsuper = NCHUNK // super_
    nbufs = 3
    z, x = ins["z"], ins["x"]
    out = outs["out"]
    has_ab = ins.get("ab_row") is not None
    has_bias = bool(ins.get("has_bias", True))

    with (
        tc.tile_pool(name="consts", bufs=1) as consts,
        tc.tile_pool(name="resident", bufs=1) as resident,
    ):
        # ---- constants ----
        ident_f = consts.tile([128, 128], F32R)
        nc.gpsimd.memset(ident_f.bitcast(F32), 0.0)
        nc.gpsimd.affine_select(
            out=ident_f, in_=ident_f,
            compare_op=mybir.AluOpType.not_equal, fill=1.0,
            base=0, pattern=[[-1, 128]], channel_multiplier=1,
        )
        ident_bf = consts.tile([128, 128], BF16)
        nc.gpsimd.memset(ident_bf, 0.0)
        nc.gpsimd.affine_select(
            out=ident_bf, in_=ident_bf,
            compare_op=mybir.AluOpType.not_equal, fill=1.0,
            base=0, pattern=[[-1, 128]], channel_multiplier=1,
        )
        ident_f8 = consts.tile([128, 128], F8)
        nc.gpsimd.memset(ident_f8, 0.0)
        nc.gpsimd.affine_select(
            out=ident_f8, in_=ident_f8,
            compare_op=mybir.AluOpType.not_equal, fill=1.0,
            base=0, pattern=[[-1, 128]], channel_multiplier=1,
        )

        # G accumulators + finalize pool live below the pass-A pools on the
        # pool stack (LIFO release order)
        gp_ctx = tc.tile_pool(name="g_psum", bufs=1, space="PSUM")
        gp = gp_ctx.__enter__()
        fin_ctx = tc.tile_pool(name="fin_sbuf", bufs=1)
        fin = fin_ctx.__enter__()
        g0 = gp.tile([128, 262], F32, tag="g0")
        g1 = gp.tile([128, 262], F32, tag="g1")
        # pass-A staging pool opens early so the first z/x loads beat the
        # small const DMAs into the (serial) DMA queue
        pa_ctx = (
            tc.tile_pool(name="pa_sbuf", bufs=4),
            tc.tile_pool(name="pa_psum", bufs=2, space="PSUM"),
            tc.tile_pool(name="pa_psum1", bufs=1, space="PSUM"),
        )
        pa, pap = pa_ctx[0].__enter__(), pa_ctx[1].__enter__()
        pap1 = pa_ctx[2].__enter__()
        z_m = ins["z"].rearrange("(s p b) f -> s p b f", p=128, b=8)
        x_m = ins["x"].rearrange("(s p b) f -> s p b f", p=128, b=8)
        stage0 = []
        zstage = pa.tile([128, 8, DIM], F8, tag="zstage")
        nc.sync.dma_start(out=zstage[:, 0:4], in_=z_m[0, :, 0:4])
        mq_s = consts.tile([128, 2, DIM], F8)
        nc.sync.dma_start(out=mq_s, in_=ins["mq"].rearrange("(t p) n -> p t n", p=128))
        xstage = pa.tile([128, 8, DIM], BF16, tag="xstage")
        nc.sync.dma_start(out=xstage[:, 0:4], in_=x_m[0, :, 0:4])
        nc.sync.dma_start(out=zstage[:, 4:8], in_=z_m[0, :, 4:8])
        nc.sync.dma_start(out=xstage[:, 4:8], in_=x_m[0, :, 4:8])
        stage0.append((zstage, xstage))
        # pre-warm the Exp activation table during the first loads
        warm = consts.tile([1, 2], BF16)
        nc.vector.memset(warm, 0.0)
        nc.scalar.activation(warm, warm, Exp)
        # -ln(16) bias column for the E/16 trick
        mln16 = consts.tile([128, 1], F32)
        nc.vector.memset(mln16, -2.772588722239781)
        # ones block-column const: PE rowsum rhs (head r-blocks -> head col)
        onesblk = consts.tile([128, 2], BF16)
        nc.gpsimd.memset(onesblk, 0.0)
        nc.vector.memset(onesblk[0:64, 0:1], 1.0)
        nc.vector.memset(onesblk[64:128, 1:2], 1.0)
        if has_ab:
            ones_row = consts.tile([1, 128], BF16)
            nc.vector.memset(ones_row, 1.0)
            ab_s = consts.tile([1, DIM], BF16)
            nc.sync.dma_start(out=ab_s, in_=ins["ab_row"])

        # ---- residents ----
        x_res = resident.tile([128, NCHUNK, XW], F8)
        xt_all = resident.tile([128, NCHUNK, 2, 128], BF16)
        et_all = resident.tile([128, NCHUNK, 2, 128], BF16)
        psbd = resident.tile([128, 2, 128], BF16)   # block-diag PS (pass-B rhs)

        if True:
            # ================= Pass A =================
            # row mapping: chunk c=(sc,j), partition p  <->  DRAM row
            # sc*128*super_ + p*super_ + j  (8KB contiguous runs per partition;
            # any bijection works because the n-pool sums over all rows and the
            # output store uses the same mapping).
            pend = []   # deferred G-matmul quads (software pipelining)

            def flush_pend(lag):
                while len(pend) > lag:
                    cq, ehq = pend.pop(0)
                    for pj in range(2):
                        c2 = cq + 2 * pj
                        ehf = ehq[:, 2 * pj : 2 * pj + 2, :, :].rearrange(
                            "p c h r -> p c (h r)"
                        )
                        for gi, g in enumerate((g0, g1)):
                            nc.tensor.matmul(
                                g[:, 0:262],
                                ehf[:, :, gi * 128 : (gi + 1) * 128],
                                x_res[:, c2 : c2 + 2, :],
                                start=(c2 == 0),
                                stop=(c2 == NCHUNK - 2),
                                perf_mode=DR,
                            )

            for sc in range(nsuper):
                if sc == 0:
                    zstage, xstage = stage0[0]
                else:
                    zstage = pa.tile([128, super_, DIM], F8, tag="zstage")
                    nc.sync.dma_start(out=zstage, in_=z_m[sc])
                    xstage = pa.tile([128, super_, DIM], BF16, tag="xstage")
                    nc.sync.dma_start(out=xstage, in_=x_m[sc])
                for cp in range(super_ // 4):
                    c = sc * super_ + 4 * cp      # first chunk of the quad
                    q4 = slice(4 * cp, 4 * cp + 4)
                    # x -> fp8 resident (G rhs; v-path); z arrives fp8
                    with nc.allow_low_precision(reason="damped v-path"):
                        nc.gpsimd.tensor_copy(
                            x_res[:, c : c + 4, 0:DIM], xstage[:, q4, :]
                        )
                    # z^T via PE transpose (fp8 transpose writes PSUM with
                    # element step 2, hence the trailing pad dim); PSUM->SBUF
                    zt_ps = pap1.tile([128, 4, 2, 128, 2], F8, tag="zt_ps")
                    for j in range(4):
                        for kt in range(2):
                            nc.tensor.transpose(
                                zt_ps[:, j, kt, :, 0],
                                zstage[:, 4 * cp + j, kt * 128 : (kt + 1) * 128],
                                ident_f8,
                            )
                    zt = pa.tile([128, 4, 2, 128], F8, tag="zt")
                    nc.vector.tensor_copy(zt, zt_ps[:, :, :, :, 0])
                    # x^T via PE transpose (bf16)
                    xt_ps = pap1.tile([128, 4, 2, 128], BF16, tag="xt_ps")
                    for j in range(4):
                        for kt in range(2):
                            nc.tensor.transpose(
                                xt_ps[:, j, kt, :],
                                xstage[:, 4 * cp + j, kt * 128 : (kt + 1) * 128],
                                ident_bf,
                            )
                    nc.scalar.copy(xt_all[:, c : c + 4], xt_ps)
                    # attn = z @ M (+ ab); pair-granularity PSUM tiles so
                    # the attn->exp chain double-buffers inside the quad
                    e_q = pa.tile([128, 4, DIM], BF16, tag="e_q")
                    for pr in range(2):
                        attn_ps = pap.tile([128, 2, DIM], F32, tag="attn_ps")
                        for jj in range(2):
                            j = 2 * pr + jj
                            nc.tensor.matmul(
                                attn_ps[:, jj, :], zt[:, j], mq_s,
                                start=True, stop=not has_ab,
                                perf_mode=DR,
                            )
                            if has_ab:
                                nc.tensor.matmul(
                                    attn_ps[:, jj, :], ones_row, ab_s,
                                    start=False, stop=True,
                                )
                        # E' = exp(attn)/16 (bf16, transient), one op per
                        # pair.  The 1/16 (bias=-ln16) keeps the rowsums
                        # inside fp8e4 range; PS is scaled x16 to match.
                        nc.scalar.activation(
                            e_q[:, 2 * pr : 2 * pr + 2, :], attn_ps, Exp,
                            bias=mln16[:, 0:1],
                        )
                    # E^T via PE transpose into resident et_all
                    et_ps = pap1.tile([128, 4, 2, 128], BF16, tag="et_ps")
                    for j in range(4):
                        for kt in range(2):
                            nc.tensor.transpose(
                                et_ps[:, j, kt, :],
                                e_q[:, j, kt * 128 : (kt + 1) * 128],
                                ident_bf,
                            )
                    nc.vector.tensor_copy(et_all[:, c : c + 4], et_ps)
                    # aux cols inside x_res: [1 | rs0..rs3 / 16 | 1]
                    # (rs stored /16 so it fits fp8e4 range; the stored colsum
                    # is then colsum/16, so sbcol is pre-scaled /16)
                    nc.gpsimd.memset(
                        bass.AP(
                            tensor=x_res.tensor,
                            offset=x_res.offset + c * XW + DIM,
                            ap=[x_res.ap[0], [XW, 4], [5, 2]],
                        ),
                        1.0,
                    )
                    aux_rs = bass.AP(
                        tensor=x_res.tensor,
                        offset=x_res.offset + c * XW + DIM + 1,
                        ap=[x_res.ap[0], [XW, 4], [1, 4]],
                    )
                    # rowsums via PE: rs[n, h] = E'^T(lhsT) @ ones_block,
                    # contraction over the hr half; lands n-partition direct
                    rs_ps = pap1.tile([128, 4, HEAD], F32, tag="rs_ps")
                    for j in range(4):
                        for kt in range(2):
                            nc.tensor.matmul(
                                rs_ps[:, j, 2 * kt : 2 * kt + 2],
                                et_all[:, c + j, kt, :],
                                onesblk,
                                start=True, stop=True,
                            )
                    with nc.allow_low_precision(reason="damped v-path"):
                        nc.vector.tensor_copy(aux_rs, rs_ps)
                        # Eh = E' * (1/rowsum') (fp8), recip + mult per quad;
                        # the mult alternates DVE/Pool to balance engines
                        rcp = pa.tile([128, 4, HEAD], F32, tag="rcp")
                        nc.vector.reciprocal(rcp, rs_ps)
                        eh = pa.tile([128, 4, HEAD, RANK], F8, tag="eh")
                        rcp_bc = bass.AP(
                            tensor=rcp.tensor,
                            offset=rcp.offset,
                            ap=[rcp.ap[0], [4, 4], [1, 4], [0, RANK]],
                        )
                        eh_eng = nc.vector if cp % 2 == 0 else nc.gpsimd
                        eh_eng.tensor_tensor(
                            out=eh,
                            in0=e_q.rearrange("p c (h r) -> p c h r", h=HEAD),
                            in1=rcp_bc,
                            op=mybir.AluOpType.mult,
                        )
                    # G += Eh^T @ [x | aux]: emit one quad LATE so the PE
                    # stream never stalls on the exp->rowsum->Eh chain.
                    pend.append((c, eh))
                    flush_pend(1)
            flush_pend(0)
            # late consts: finalize/pass-B parameters load after the big
            # pass-A streams have drained the DMA queue
            swv_s = consts.tile([128, 2, DIM], BF16)
            nc.sync.dma_start(
                out=swv_s, in_=ins["swv"].rearrange("(t p) n -> p t n", p=128)
            )
            wv_s = consts.tile([128, 2, DIM], F32R)
            nc.sync.dma_start(
                out=wv_s, in_=ins["wv"].rearrange("(t p) n -> p t n", p=128)
            )
            bvp_bc = consts.tile([128, DIM], F32)
            nc.gpsimd.dma_start(
                out=bvp_bc, in_=ins["bv_row"].to_broadcast([128, DIM])
            )
            if has_bias:
                biasout_bc = consts.tile([128, DIM], F32)
                nc.gpsimd.dma_start(
                    out=biasout_bc, in_=ins["biasout_row"].to_broadcast([128, DIM])
                )
            sbcol_s = consts.tile([128, 2], F32)
            nc.sync.dma_start(out=sbcol_s, in_=ins["sbcol"])
            pa_ctx[2].__exit__(None, None, None)
            pa_ctx[1].__exit__(None, None, None)
            pa_ctx[0].__exit__(None, None, None)

            # ================= Finalize =================
            finp_ctx = tc.tile_pool(name="fin_psum", bufs=1, space="PSUM")
            finp = finp_ctx.__enter__()
            for gi, g in enumerate((g0, g1)):
                gs = fin.tile([128, 262], F32R, tag=f"gs{gi}")
                if gi == 0:
                    nc.vector.tensor_copy(gs, g)
                else:
                    nc.scalar.copy(gs, g)
                gt_ps = finp.tile([128, 2, 128], F32R, tag="gt_ps")
                for kt in range(2):
                    nc.tensor.transpose(
                        gt_ps[:, kt, :],
                        gs[:, kt * 128 : (kt + 1) * 128],
                        ident_f,
                    )
                gt = fin.tile([128, 2, 128], F32R, tag=f"gt{gi}")
                if gi == 0:
                    nc.vector.tensor_copy(gt, gt_ps)
                else:
                    nc.scalar.copy(gt, gt_ps)
                p_ps = finp.tile([128, 128], F32, tag="p_ps")
                for kt in range(2):
                    nc.tensor.matmul(
                        p_ps,
                        gt[:, kt, :],
                        wv_s[:, kt, gi * 128 : (gi + 1) * 128],
                        start=(kt == 0), stop=(kt == 1),
                    )
                # pooled = p_ps + esum * bv
                pool_s = fin.tile([128, 128], F32, tag=f"pool_s{gi}")
                nc.vector.scalar_tensor_tensor(
                    out=pool_s,
                    in0=bvp_bc[:, gi * 128 : (gi + 1) * 128],
                    scalar=gs[:, 256:257],
                    in1=p_ps,
                    op0=mybir.AluOpType.mult,
                    op1=mybir.AluOpType.add,
                )
                # colsum (col 257 for even head rows, 258 for odd head rows)
                cs = fin.tile([128, 1], F32, tag=f"cs{gi}")
                h0, h1 = 2 * gi, 2 * gi + 1
                nc.vector.tensor_copy(cs[0:64, :], gs[0:64, 257 + h0 : 258 + h0])
                nc.vector.tensor_copy(cs[64:128, :], gs[64:128, 257 + h1 : 258 + h1])
                rcs = fin.tile([128, 1], F32, tag=f"rcs{gi}")
                nc.vector.reciprocal(rcs, cs)
                nc.vector.tensor_mul(rcs, rcs, sbcol_s[:, gi : gi + 1])
                # PS block-diag (bf16): rows = this pair's (h even r | h odd r)
                if gi == 0:
                    nc.gpsimd.memset(psbd, 0.0)
                nc.vector.tensor_scalar_mul(
                    psbd[0:64, gi, 0:64], pool_s[0:64, 0:64], rcs[0:64, :]
                )
                nc.vector.tensor_scalar_mul(
                    psbd[64:128, gi, 64:128], pool_s[64:128, 64:128], rcs[64:128, :]
                )

            finp_ctx.__exit__(None, None, None)
            fin_ctx.__exit__(None, None, None)
            gp_ctx.__exit__(None, None, None)

        # ================= Pass B =================
        with (
            tc.tile_pool(name="pb_sbuf", bufs=nbufs) as pb,
            tc.tile_pool(name="pb_psum", bufs=2, space="PSUM") as pbp,
        ):
            o_m = out.rearrange("(s p b) f -> s p b f", p=128, b=super_)
            for sc in range(nsuper):
                ostage = pb.tile([128, super_, DIM], BF16, tag="ostage")
                for cp in range(super_ // 2):
                    c = sc * super_ + 2 * cp
                    out_ps = pbp.tile([128, 2, DIM], F32, tag="out_ps")
                    for j in range(2):
                        nc.tensor.matmul(
                            out_ps[:, j, :], xt_all[:, c + j, 0, :], swv_s[:, 0, :],
                            start=True, stop=False,
                        )
                        nc.tensor.matmul(
                            out_ps[:, j, :], xt_all[:, c + j, 1, :], swv_s[:, 1, :],
                            start=False, stop=False,
                        )
                        nc.tensor.matmul(
                            out_ps[:, j, 0:128], et_all[:, c + j, 0, :], psbd[:, 0, :],
                            start=False, stop=False,
                        )
                        nc.tensor.matmul(
                            out_ps[:, j, 128:256], et_all[:, c + j, 1, :],
                            psbd[:, 1, :],
                            start=False, stop=True,
                        )
                    # out = psum (+ bias); engine alternates for balance
                    if has_bias:
                        bias_bc2 = bass.AP(
                            tensor=biasout_bc.tensor,
                            offset=biasout_bc.offset,
                            ap=[biasout_bc.ap[0], [0, 2], [1, DIM]],
                        )
                        nc.vector.tensor_add(
                            ostage[:, 2 * cp : 2 * cp + 2, :], out_ps, bias_bc2
                        )
                    elif cp % 2 == 0:
                        nc.vector.tensor_copy(
                            ostage[:, 2 * cp : 2 * cp + 2, :], out_ps
                        )
                    else:
                        nc.scalar.copy(ostage[:, 2 * cp : 2 * cp + 2, :], out_ps)
                nc.sync.dma_start(out=o_m[sc, :, 0:4], in_=ostage[:, 0:4])
                nc.sync.dma_start(out=o_m[sc, :, 4:8], in_=ostage[:, 4:8])


def fold_params(Wq, bq, K, Wv, bv, alpha, beta):
    """Host-side folding of the tiny parameter tensors (all O(256^2))."""
    Wq = np.asarray(Wq, np.float64)
    bq = np.asarray(bq, np.float64)
    K = np.asarray(K, np.float64)
    Wv = np.asarray(Wv, np.float64)
    bv = np.asarray(bv, np.float64)
    sa = 1.0 / (1.0 + np.exp(-np.asarray(alpha, np.float64)[:, 0]))  # (HEAD,)
    sb = 1.0 / (1.0 + np.exp(-np.asarray(beta, np.float64)[:, 0]))
    scale = 1.0 / math.sqrt(HDIM)
    # M[:, h*RANK + r] = Wq_h @ K_h^T / sqrt(d)
    M = np.zeros((DIM, HEAD * RANK))
    ab = np.zeros((HEAD * RANK,))
    for h in range(HEAD):
        Kh = K[:, h, :]  # (RANK, HDIM)
        M[:, h * RANK : (h + 1) * RANK] = (
            Wq[:, h * HDIM : (h + 1) * HDIM] @ Kh.T * scale
        )
        ab[h * RANK : (h + 1) * RANK] = (bq[h * HDIM : (h + 1) * HDIM] @ Kh.T) * scale
    sa_vec = np.repeat(sa, HDIM)  # (256,)
    swv = Wv * sa_vec[None, :]
    biasout = bv * sa_vec
    sbcol = np.zeros((128, 2))
    for gi in range(2):
        sbcol[0:64, gi] = sb[2 * gi] / 16.0
        sbcol[64:128, gi] = sb[2 * gi + 1] / 16.0
    return {
        "mq": M.astype(np.float32),
        "ab": ab.astype(np.float32),
        "swv": swv.astype(np.float32),
        "wv": Wv.astype(np.float32),
        "bv_row": bv.astype(np.float32).reshape(1, DIM),
        "biasout_row": biasout.astype(np.float32).reshape(1, DIM),
        "sbcol": sbcol.astype(np.float32),
    }


PK_LAYOUT = {  # name -> (byte offset, dtype, logical shape); 64B-aligned
    "mq": (0, F8, (DIM, DIM)),                 # 65536 B
    "swv": (65536, BF16, (DIM, DIM)),          # 131072 B
    "wv": (196608, F32R, (DIM, DIM)),          # 262144 B
    "bv_row": (458752, F32, (1, DIM)),         # 1024 B
    "biasout_row": (459776, F32, (1, DIM)),    # 1024 B
    "sbcol": (460800, F32, (128, 2)),          # 1024 B
    "ab_row": (461824, BF16, (1, DIM)),        # 512 B
}
PK_BYTES = 462336


ZBYTES = N * DIM          # z as fp8e4, 1 B/elem
XBYTES = N * DIM * 2      # x as bf16
ZX_BYTES = ZBYTES + XBYTES


def build_nc(has_ab, has_bias=True):
    nc = bacc.Bacc("TRN2", target_bir_lowering=False, debug=False,
                   enable_asserts=False)
    zxb = nc.dram_tensor("zx", [ZX_BYTES], mybir.dt.uint8, kind="ExternalInput").ap()
    pk = nc.dram_tensor("pk", [PK_BYTES], mybir.dt.uint8, kind="ExternalInput").ap()

    def pk_view(name):
        off, dt, shape = PK_LAYOUT[name]
        esz = mybir.dt.size(dt)
        flat = pk[off : off + esz * shape[0] * shape[1]].bitcast(dt)
        return flat.rearrange("(a b) -> a b", a=shape[0])

    ins = {
        "z": zxb[0:ZBYTES].bitcast(F8).rearrange("(a b) -> a b", a=N),
        "x": zxb[ZBYTES:ZX_BYTES].bitcast(BF16).rearrange("(a b) -> a b", a=N),
        "mq": pk_view("mq"),
        "swv": pk_view("swv"),
        "wv": pk_view("wv"),
        "bv_row": pk_view("bv_row"),
        "biasout_row": pk_view("biasout_row"),
        "sbcol": pk_view("sbcol"),
        "ab_row": pk_view("ab_row") if has_ab else None,
    }
    ins["has_bias"] = has_bias
    outs = {"out": nc.dram_tensor("out", [N, DIM], BF16, kind="ExternalOutput").ap()}
    reps = int(os.environ.get("KREPS", "1"))
    with tile.TileContext(nc) as tc:
        for _ in range(reps):
            build_body(tc, outs, ins)
    nc.compile()
    return nc


LAST_RESULTS = None


def pack_zx(z_core, x_core):
    """Host-side wire packing: z as fp8e4 (the attn path quantizes to fp8
    on-device anyway), x as bf16 (its consumers are bf16/fp8).  Halves+
    HBM load traffic 16 MB -> 6 MB per core."""
    import ml_dtypes

    zb = np.ascontiguousarray(
        z_core.astype(ml_dtypes.float8_e4m3)
    ).view(np.uint8).reshape(-1)
    xb = np.ascontiguousarray(
        x_core.astype(ml_dtypes.bfloat16)
    ).view(np.uint8).reshape(-1)
    return np.concatenate([zb, xb])


def pack_params(p, has_ab):
    """Byte-pack the folded params per PK_LAYOUT into one uint8 tensor."""
    import ml_dtypes

    vals = {
        "mq": p["mq"].astype(ml_dtypes.float8_e4m3),
        "swv": p["swv"].astype(ml_dtypes.bfloat16),
        "wv": p["wv"].astype(np.float32),
        "bv_row": p["bv_row"].astype(np.float32),
        "biasout_row": p["biasout_row"].astype(np.float32),
        "sbcol": p["sbcol"].astype(np.float32),
    }
    if has_ab:
        vals["ab_row"] = p["ab"].reshape(1, DIM).astype(ml_dtypes.bfloat16)
    pk = np.zeros(PK_BYTES, np.uint8)
    for name, arr in vals.items():
        off = PK_LAYOUT[name][0]
        b = np.ascontiguousarray(arr).view(np.uint8).reshape(-1)
        pk[off : off + b.size] = b
    return pk


def kernel(x, z, Wq, bq, K, Wv, bv, alpha, beta):
    global LAST_RESULTS
    import ml_dtypes
    from concourse.bass_utils import run_bass_kernel_spmd

    x = np.ascontiguousarray(np.asarray(x, np.float32))
    z = np.ascontiguousarray(np.asarray(z, np.float32))
    p = fold_params(Wq, bq, K, Wv, bv, alpha, beta)
    has_ab = bool(np.any(p["ab"] != 0.0))
    has_bias = bool(np.any(p["biasout_row"] != 0.0))

    nc = build_nc(has_ab, has_bias)

    pk = pack_params(p, has_ab)
    in_maps = [
        {"zx": pack_zx(z[i], x[i]), "pk": pk} for i in range(NCORES)
    ]
    res = run_bass_kernel_spmd(nc, in_maps, core_ids=list(range(NCORES)))
    LAST_RESULTS = res
    out = np.stack([res.results[i]["out"] for i in range(NCORES)], axis=0)
    return out.astype(np.float32)


# revision 39
# speedup vs baseline: 2.3181x; 2.3181x over previous
# Trainium2 Bass kernel for nn_Lowrank_Spattention (sparse_attention).
#
# Reference math (per batch b, n=8192 tokens, f=256 features, h=4 heads,
# r=64 latent ranks, d=64 head dim):
#   q    = z @ Wq + bq                    (n, h*d)
#   attn = einsum(q, K)/sqrt(d)           (n, h*r)   == z @ M + ab
#            where M[:, h*r+j] = (Wq_h @ K_h^T)/8,  ab = bq @ K^T/8
#   xv   = x @ Wv + bv                    (n, h*d)
#   pooled = softmax_r(attn)^T-pool of xv (r, h*d)
#   v    = softmax_n(attn) @ pooled       (n, h*d)
#   out  = sig(alpha)*xv + sig(beta)*v
#
# Kernel strategy (one NeuronCore per batch element, 8 cores, no
# collectives; inputs packed into two tensors, zx = [z;x] and pk =
# byte-packed params, to minimize per-launch buffer marshaling):
#
#   Pass A is DMA-bound (16 MB of z+x loads on the serial DMA bus), so
#   ALL PE work beyond attn runs in the DMA shadow.  Per 128-row chunk:
#     zt   z^T (fp8, transient; fp8 transpose writes PSUM at element
#          step 2, copied compact to SBUF)
#     attn = zt^T @ mq as ONE fp8 DoubleRow matmul (k-halves ride the
#          [K,2,N] APs; both operands use the same (p,kt) mapping)
#     E'   = exp(attn - ln16) (bf16; the /16 keeps rowsums in fp8e4
#          range, PS is rescaled in finalize to compensate)
#     et   E'^T (bf16, resident)      for pass B's  E' @ PS_bd
#     xt   x^T (bf16, resident)       for pass B's  x @ sig(a)Wv
#          (transposed from f32r, the PSUM->SBUF copy downcasts)
#     x_res x (fp8) + aux cols [1 | rowsums' | 1] (fp8, resident)
#     G += Eh^T @ [x|aux] as fp8 DoubleRow matmuls over chunk PAIRS,
#          accumulated in PSUM over all 8192 rows; Eh = E'/rowsum'.
#   Finalize (tiny): pooled = G[:, :256] @ Wv + esum*bv;
#     PS = 16 * sig(beta) * pooled / colsum, block-diagonal (bf16).
#   Pass B is a pure matmul stream + store (PE-light, store-DMA-bound):
#     out = xt^T @ (sig(alpha)Wv) + et^T @ PS_bd (+ bias).
#
# The whole v-path (E, G, pooled) is fp8/bf16: its output contribution
# is scaled by sig(beta)=0.01 and pooled averages 8192 rows, damping
# its relative error ~1e2-1e4x.  The xv-path runs bf16 into f32 PSUM
# (~1e-3 rel err on out; tolerance is 2e-2).  Measured sim rel err
# 2.5e-3, CoreSim model time ~95 us/core vs a ~85 us DMA-floor
# (24 MB of HBM traffic at ~345 GB/s + startup/finalize/tail).
#
# Engine balance per 4-chunk quad in pass A (model): DMA 3.16 us,
# PE ~1.8, DVE ~2.7 (zt/et copies, rowsum reduce, Eh mult alt.),
# Act ~2.5 (exp, xt copies, et alt.), Pool ~2.2 (casts, Eh alt.).

import math
import os

import numpy as np

import concourse.bass as bass
import concourse.mybir as mybir
import concourse.tile as tile
from concourse import bacc

B, N, DIM = 8, 8192, 256
HEAD, RANK, HDIM = 4, 64, 64
NCORES = 8
CHUNK = 128                 # rows per compute chunk
NCHUNK = N // CHUNK         # 64
XW = DIM + 6                # x_res row width: 256 x cols + [1|rs0..3|1]

F32 = mybir.dt.float32
F32R = mybir.dt.float32r
BF16 = mybir.dt.bfloat16
F8 = mybir.dt.float8e4
Exp = mybir.ActivationFunctionType.Exp
DR = mybir.MatmulPerfMode.DoubleRow


def build_body(tc, outs, ins):
    """Emit the per-core program.  outs/ins are dicts of bass.APs."""
    nc = tc.nc
    super_ = 8                  # chunks per staged DMA
    nsuper = NCHUNK // super_
    nbufs = 3
    z, x = ins["z"], ins["x"]
    out = outs["out"]
    has_ab = ins.get("ab_row") is not None
    has_bias = bool(ins.get("has_bias", True))

    with (
        tc.tile_pool(name="consts", bufs=1) as consts,
        tc.tile_pool(name="resident", bufs=1) as resident,
    ):
        # ---- constants ----
        ident_f = consts.tile([128, 128], F32R)
        nc.gpsimd.memset(ident_f.bitcast(F32), 0.0)
        nc.gpsimd.affine_select(
            out=ident_f, in_=ident_f,
            compare_op=mybir.AluOpType.not_equal, fill=1.0,
            base=0, pattern=[[-1, 128]], channel_multiplier=1,
        )
        ident_bf = consts.tile([128, 128], BF16)
        nc.gpsimd.memset(ident_bf, 0.0)
        nc.gpsimd.affine_select(
            out=ident_bf, in_=ident_bf,
            compare_op=mybir.AluOpType.not_equal, fill=1.0,
            base=0, pattern=[[-1, 128]], channel_multiplier=1,
        )
        ident_f8 = consts.tile([128, 128], F8)
        nc.gpsimd.memset(ident_f8, 0.0)
        nc.gpsimd.affine_select(
            out=ident_f8, in_=ident_f8,
            compare_op=mybir.AluOpType.not_equal, fill=1.0,
            base=0, pattern=[[-1, 128]], channel_multiplier=1,
        )

        # G accumulators + finalize pool live below the pass-A pools on the
        # pool stack (LIFO release order)
        gp_ctx = tc.tile_pool(name="g_psum", bufs=1, space="PSUM")
        gp = gp_ctx.__enter__()
        fin_ctx = tc.tile_pool(name="fin_sbuf", bufs=1)
        fin = fin_ctx.__enter__()
        g0 = gp.tile([128, 262], F32, tag="g0")
        g1 = gp.tile([128, 262], F32, tag="g1")
        # pass-A staging pool opens early so the first z/x loads beat the
        # small const DMAs into the (serial) DMA queue
        pa_ctx = (
            tc.tile_pool(name="pa_sbuf", bufs=4),
            tc.tile_pool(name="pa_psum", bufs=2, space="PSUM"),
            tc.tile_pool(name="pa_psum1", bufs=1, space="PSUM"),
        )
        pa, pap = pa_ctx[0].__enter__(), pa_ctx[1].__enter__()
        pap1 = pa_ctx[2].__enter__()
        z_m = ins["z"].rearrange("(s p b) f -> s p b f", p=128, b=8)
        x_m = ins["x"].rearrange("(s p b) f -> s p b f", p=128, b=8)
        stage0 = []
        zstage = pa.tile([128, 8, DIM], F8, tag="zstage")
        nc.sync.dma_start(out=zstage[:, 0:4], in_=z_m[0, :, 0:4])
        mq_s = consts.tile([128, 2, DIM], F8)
        nc.sync.dma_start(out=mq_s, in_=ins["mq"].rearrange("(t p) n -> p t n", p=128))
        xstage = pa.tile([128, 8, DIM], BF16, tag="xstage")
        nc.sync.dma_start(out=xstage[:, 0:4], in_=x_m[0, :, 0:4])
        nc.sync.dma_start(out=zstage[:, 4:8], in_=z_m[0, :, 4:8])
        nc.sync.dma_start(out=xstage[:, 4:8], in_=x_m[0, :, 4:8])
        stage0.append((zstage, xstage))
        # pre-warm the Exp activation table during the first loads
        warm = consts.tile([1, 2], BF16)
        nc.vector.memset(warm, 0.0)
        nc.scalar.activation(warm, warm, Exp)
        # -ln(16) bias column for the E/16 trick
        mln16 = consts.tile([128, 1], F32)
        nc.vector.memset(mln16, -2.772588722239781)
        # ones block-column const: PE rowsum rhs (head r-blocks -> head col)
        onesblk = consts.tile([128, 2], BF16)
        nc.gpsimd.memset(onesblk, 0.0)
        nc.vector.memset(onesblk[0:64, 0:1], 1.0)
        nc.vector.memset(onesblk[64:128, 1:2], 1.0)
        if has_ab:
            ones_row = consts.tile([1, 128], BF16)
            nc.vector.memset(ones_row, 1.0)
            ab_s = consts.tile([1, DIM], BF16)
            nc.sync.dma_start(out=ab_s, in_=ins["ab_row"])

        # ---- residents ----
        x_res = resident.tile([128, NCHUNK, XW], F8)
        xt_all = resident.tile([128, NCHUNK, 2, 128], BF16)
        et_all = resident.tile([128, NCHUNK, 2, 128], BF16)
        psbd = resident.tile([128, 2, 128], BF16)   # block-diag PS (pass-B rhs)

        if True:
            # ================= Pass A =================
            # row mapping: chunk c=(sc,j), partition p  <->  DRAM row
            # sc*128*super_ + p*super_ + j  (8KB contiguous runs per partition;
            # any bijection works because the n-pool sums over all rows and the
            # output store uses the same mapping).
            pend = []   # deferred G-matmul quads (software pipelining)

            def flush_pend(lag):
                while len(pend) > lag:
                    cq, ehq = pend.pop(0)
                    for pj in range(2):
                        c2 = cq + 2 * pj
                        ehf = ehq[:, 2 * pj : 2 * pj + 2, :, :].rearrange(
                            "p c h r -> p c (h r)"
                        )
                        for gi, g in enumerate((g0, g1)):
                            nc.tensor.matmul(
                                g[:, 0:262],
                                ehf[:, :, gi * 128 : (gi + 1) * 128],
                                x_res[:, c2 : c2 + 2, :],
                                start=(c2 == 0),
                                stop=(c2 == NCHUNK - 2),
                                perf_mode=DR,
                            )

            for sc in range(nsuper):
                if sc == 0:
                    zstage, xstage = stage0[0]
                else:
                    zstage = pa.tile([128, super_, DIM], F8, tag="zstage")
                    nc.sync.dma_start(out=zstage, in_=z_m[sc])
                    xstage = pa.tile([128, super_, DIM], BF16, tag="xstage")
                    nc.sync.dma_start(out=xstage, in_=x_m[sc])
                for cp in range(super_ // 4):
                    c = sc * super_ + 4 * cp      # first chunk of the quad
                    q4 = slice(4 * cp, 4 * cp + 4)
                    # x -> fp8 resident (G rhs; v-path); z arrives fp8
                    with nc.allow_low_precision(reason="damped v-path"):
                        nc.gpsimd.tensor_copy(
                            x_res[:, c : c + 4, 0:DIM], xstage[:, q4, :]
                        )
                    # z^T via PE transpose (fp8 transpose writes PSUM with
                    # element step 2, hence the trailing pad dim); PSUM->SBUF
                    zt_ps = pap1.tile([128, 4, 2, 128, 2], F8, tag="zt_ps")
                    for j in range(4):
                        for kt in range(2):
                            nc.tensor.transpose(
                                zt_ps[:, j, kt, :, 0],
                                zstage[:, 4 * cp + j, kt * 128 : (kt + 1) * 128],
                                ident_f8,
                            )
                    # copy the stride-2 fp8 PSUM region contiguously as
                    # bf16 (2x DVE mode); attn reads a stride-2 fp8 view
                    zt_w = pa.tile([128, 4, 2, 128], BF16, tag="zt")
                    nc.vector.tensor_copy(zt_w, zt_ps.bitcast(BF16))
                    # x^T via PE transpose (bf16)
                    xt_ps = pap1.tile([128, 4, 2, 128], BF16, tag="xt_ps")
                    for j in range(4):
                        for kt in range(2):
                            nc.tensor.transpose(
                                xt_ps[:, j, kt, :],
                                xstage[:, 4 * cp + j, kt * 128 : (kt + 1) * 128],
                                ident_bf,
                            )
                    nc.scalar.copy(xt_all[:, c : c + 4], xt_ps)
                    # attn = z @ M (+ ab); pair-granularity PSUM tiles so
                    # the attn->exp chain double-buffers inside the quad
                    e_q = pa.tile([128, 4, DIM], BF16, tag="e_q")
                    for pr in range(2):
                        attn_ps = pap.tile([128, 2, DIM], F32, tag="attn_ps")
                        for jj in range(2):
                            j = 2 * pr + jj
                            ztv = zt_w[:, j].bitcast(F8)
                            ztv = bass.AP(
                                tensor=ztv.tensor, offset=ztv.offset,
                                ap=[ztv.ap[0], ztv.ap[1], [2, 128]],
                            )
                            nc.tensor.matmul(
                                attn_ps[:, jj, :], ztv, mq_s,
                                start=True, stop=not has_ab,
                                perf_mode=DR,
                            )
                            if has_ab:
                                nc.tensor.matmul(
                                    attn_ps[:, jj, :], ones_row, ab_s,
                                    start=False, stop=True,
                                )
                        # E' = exp(attn)/16 (bf16, transient), one op per
                        # pair.  The 1/16 (bias=-ln16) keeps the rowsums
                        # inside fp8e4 range; PS is scaled x16 to match.
                        nc.scalar.activation(
                            e_q[:, 2 * pr : 2 * pr + 2, :], attn_ps, Exp,
                            bias=mln16[:, 0:1],
                        )
                    # E^T via PE transpose into resident et_all
                    et_ps = pap1.tile([128, 4, 2, 128], BF16, tag="et_ps")
                    for j in range(4):
                        for kt in range(2):
                            nc.tensor.transpose(
                                et_ps[:, j, kt, :],
                                e_q[:, j, kt * 128 : (kt + 1) * 128],
                                ident_bf,
                            )
                    nc.vector.tensor_copy(et_all[:, c : c + 4], et_ps)
                    # aux cols inside x_res: [1 | rs0..rs3 / 16 | 1]
                    # (rs stored /16 so it fits fp8e4 range; the stored colsum
                    # is then colsum/16, so sbcol is pre-scaled /16)
                    nc.gpsimd.memset(
                        bass.AP(
                            tensor=x_res.tensor,
                            offset=x_res.offset + c * XW + DIM,
                            ap=[x_res.ap[0], [XW, 4], [5, 2]],
                        ),
                        1.0,
                    )
                    aux_rs = bass.AP(
                        tensor=x_res.tensor,
                        offset=x_res.offset + c * XW + DIM + 1,
                        ap=[x_res.ap[0], [XW, 4], [1, 4]],
                    )
                    # rowsums via PE: rs[n, h] = E'^T(lhsT) @ ones_block,
                    # contraction over the hr half; lands n-partition direct
                    rs_ps = pap1.tile([128, 4, HEAD], F32, tag="rs_ps")
                    for j in range(4):
                        for kt in range(2):
                            nc.tensor.matmul(
                                rs_ps[:, j, 2 * kt : 2 * kt + 2],
                                et_all[:, c + j, kt, :],
                                onesblk,
                                start=True, stop=True,
                            )
                    with nc.allow_low_precision(reason="damped v-path"):
                        nc.vector.tensor_copy(aux_rs, rs_ps)
                        # Eh = E' * (1/rowsum') (fp8), recip + mult per quad;
                        # the mult alternates DVE/Pool to balance engines
                        rcp = pa.tile([128, 4, HEAD], F32, tag="rcp")
                        nc.vector.reciprocal(rcp, rs_ps)
                        eh = pa.tile([128, 4, HEAD, RANK], F8, tag="eh")
                        rcp_bc = bass.AP(
                            tensor=rcp.tensor,
                            offset=rcp.offset,
                            ap=[rcp.ap[0], [4, 4], [1, 4], [0, RANK]],
                        )
                        eh_eng = nc.vector if cp % 2 == 0 else nc.gpsimd
                        eh_eng.tensor_tensor(
                            out=eh,
                            in0=e_q.rearrange("p c (h r) -> p c h r", h=HEAD),
                            in1=rcp_bc,
                            op=mybir.AluOpType.mult,
                        )
                    # G += Eh^T @ [x | aux]: emit one quad LATE so the PE
                    # stream never stalls on the exp->rowsum->Eh chain.
                    pend.append((c, eh))
                    flush_pend(1)
            flush_pend(0)
            # late consts: finalize/pass-B parameters load after the big
            # pass-A streams have drained the DMA queue
            swv_s = consts.tile([128, 2, DIM], BF16)
            nc.sync.dma_start(
                out=swv_s, in_=ins["swv"].rearrange("(t p) n -> p t n", p=128)
            )
            wv_s = consts.tile([128, 2, DIM], F32R)
            nc.sync.dma_start(
                out=wv_s, in_=ins["wv"].rearrange("(t p) n -> p t n", p=128)
            )
            bvp_bc = consts.tile([128, DIM], F32)
            nc.gpsimd.dma_start(
                out=bvp_bc, in_=ins["bv_row"].to_broadcast([128, DIM])
            )
            if has_bias:
                biasout_bc = consts.tile([128, DIM], F32)
                nc.gpsimd.dma_start(
                    out=biasout_bc, in_=ins["biasout_row"].to_broadcast([128, DIM])
                )
            sbcol_s = consts.tile([128, 2], F32)
            nc.sync.dma_start(out=sbcol_s, in_=ins["sbcol"])
            pa_ctx[2].__exit__(None, None, None)
            pa_ctx[1].__exit__(None, None, None)
            pa_ctx[0].__exit__(None, None, None)

            # ================= Finalize =================
            finp_ctx = tc.tile_pool(name="fin_psum", bufs=1, space="PSUM")
            finp = finp_ctx.__enter__()
            for gi, g in enumerate((g0, g1)):
                gs = fin.tile([128, 262], F32R, tag=f"gs{gi}")
                if gi == 0:
                    nc.vector.tensor_copy(gs, g)
                else:
                    nc.scalar.copy(gs, g)
                gt_ps = finp.tile([128, 2, 128], F32R, tag="gt_ps")
                for kt in range(2):
                    nc.tensor.transpose(
                        gt_ps[:, kt, :],
                        gs[:, kt * 128 : (kt + 1) * 128],
                        ident_f,
                    )
                gt = fin.tile([128, 2, 128], F32R, tag=f"gt{gi}")
                if gi == 0:
                    nc.vector.tensor_copy(gt, gt_ps)
                else:
                    nc.scalar.copy(gt, gt_ps)
                p_ps = finp.tile([128, 128], F32, tag="p_ps")
                for kt in range(2):
                    nc.tensor.matmul(
                        p_ps,
                        gt[:, kt, :],
                        wv_s[:, kt, gi * 128 : (gi + 1) * 128],
                        start=(kt == 0), stop=(kt == 1),
                    )
                # pooled = p_ps + esum * bv
                pool_s = fin.tile([128, 128], F32, tag=f"pool_s{gi}")
                nc.vector.scalar_tensor_tensor(
                    out=pool_s,
                    in0=bvp_bc[:, gi * 128 : (gi + 1) * 128],
                    scalar=gs[:, 256:257],
                    in1=p_ps,
                    op0=mybir.AluOpType.mult,
                    op1=mybir.AluOpType.add,
                )
                # colsum (col 257 for even head rows, 258 for odd head rows)
                cs = fin.tile([128, 1], F32, tag=f"cs{gi}")
                h0, h1 = 2 * gi, 2 * gi + 1
                nc.vector.tensor_copy(cs[0:64, :], gs[0:64, 257 + h0 : 258 + h0])
                nc.vector.tensor_copy(cs[64:128, :], gs[64:128, 257 + h1 : 258 + h1])
                rcs = fin.tile([128, 1], F32, tag=f"rcs{gi}")
                nc.vector.reciprocal(rcs, cs)
                nc.vector.tensor_mul(rcs, rcs, sbcol_s[:, gi : gi + 1])
                # PS block-diag (bf16): rows = this pair's (h even r | h odd r)
                if gi == 0:
                    nc.gpsimd.memset(psbd, 0.0)
                nc.vector.tensor_scalar_mul(
                    psbd[0:64, gi, 0:64], pool_s[0:64, 0:64], rcs[0:64, :]
                )
                nc.vector.tensor_scalar_mul(
                    psbd[64:128, gi, 64:128], pool_s[64:128, 64:128], rcs[64:128, :]
                )

            finp_ctx.__exit__(None, None, None)
            fin_ctx.__exit__(None, None, None)
            gp_ctx.__exit__(None, None, None)

        # ================= Pass B =================
        with (
            tc.tile_pool(name="pb_sbuf", bufs=nbufs) as pb,
            tc.tile_pool(name="pb_psum", bufs=2, space="PSUM") as pbp,
        ):
            o_m = out.rearrange("(s p b) f -> s p b f", p=128, b=super_)
            for sc in range(nsuper):
                ostage = pb.tile([128, super_, DIM], BF16, tag="ostage")
                for cp in range(super_ // 2):
                    c = sc * super_ + 2 * cp
                    out_ps = pbp.tile([128, 2, DIM], F32, tag="out_ps")
                    for j in range(2):
                        nc.tensor.matmul(
                            out_ps[:, j, :], xt_all[:, c + j, 0, :], swv_s[:, 0, :],
                            start=True, stop=False,
                        )
                        nc.tensor.matmul(
                            out_ps[:, j, :], xt_all[:, c + j, 1, :], swv_s[:, 1, :],
                            start=False, stop=False,
                        )
                        nc.tensor.matmul(
                            out_ps[:, j, 0:128], et_all[:, c + j, 0, :], psbd[:, 0, :],
                            start=False, stop=False,
                        )
                        nc.tensor.matmul(
                            out_ps[:, j, 128:256], et_all[:, c + j, 1, :],
                            psbd[:, 1, :],
                            start=False, stop=True,
                        )
                    # out = psum (+ bias); engine alternates for balance
                    if has_bias:
                        bias_bc2 = bass.AP(
                            tensor=biasout_bc.tensor,
                            offset=biasout_bc.offset,
                            ap=[biasout_bc.ap[0], [0, 2], [1, DIM]],
                        )
                        nc.vector.tensor_add(
                            ostage[:, 2 * cp : 2 * cp + 2, :], out_ps, bias_bc2
                        )
                    elif cp % 2 == 0:
                        nc.vector.tensor_copy(
                            ostage[:, 2 * cp : 2 * cp + 2, :], out_ps
                        )
                    else:
                        nc.scalar.copy(ostage[:, 2 * cp : 2 * cp + 2, :], out_ps)
                nc.sync.dma_start(out=o_m[sc, :, 0:4], in_=ostage[:, 0:4])
                nc.sync.dma_start(out=o_m[sc, :, 4:8], in_=ostage[:, 4:8])


def fold_params(Wq, bq, K, Wv, bv, alpha, beta):
    """Host-side folding of the tiny parameter tensors (all O(256^2))."""
    Wq = np.asarray(Wq, np.float64)
    bq = np.asarray(bq, np.float64)
    K = np.asarray(K, np.float64)
    Wv = np.asarray(Wv, np.float64)
    bv = np.asarray(bv, np.float64)
    sa = 1.0 / (1.0 + np.exp(-np.asarray(alpha, np.float64)[:, 0]))  # (HEAD,)
    sb = 1.0 / (1.0 + np.exp(-np.asarray(beta, np.float64)[:, 0]))
    scale = 1.0 / math.sqrt(HDIM)
    # M[:, h*RANK + r] = Wq_h @ K_h^T / sqrt(d)
    M = np.zeros((DIM, HEAD * RANK))
    ab = np.zeros((HEAD * RANK,))
    for h in range(HEAD):
        Kh = K[:, h, :]  # (RANK, HDIM)
        M[:, h * RANK : (h + 1) * RANK] = (
            Wq[:, h * HDIM : (h + 1) * HDIM] @ Kh.T * scale
        )
        ab[h * RANK : (h + 1) * RANK] = (bq[h * HDIM : (h + 1) * HDIM] @ Kh.T) * scale
    sa_vec = np.repeat(sa, HDIM)  # (256,)
    swv = Wv * sa_vec[None, :]
    biasout = bv * sa_vec
    sbcol = np.zeros((128, 2))
    for gi in range(2):
        sbcol[0:64, gi] = sb[2 * gi] / 16.0
        sbcol[64:128, gi] = sb[2 * gi + 1] / 16.0
    return {
        "mq": M.astype(np.float32),
        "ab": ab.astype(np.float32),
        "swv": swv.astype(np.float32),
        "wv": Wv.astype(np.float32),
        "bv_row": bv.astype(np.float32).reshape(1, DIM),
        "biasout_row": biasout.astype(np.float32).reshape(1, DIM),
        "sbcol": sbcol.astype(np.float32),
    }


PK_LAYOUT = {  # name -> (byte offset, dtype, logical shape); 64B-aligned
    "mq": (0, F8, (DIM, DIM)),                 # 65536 B
    "swv": (65536, BF16, (DIM, DIM)),          # 131072 B
    "wv": (196608, F32R, (DIM, DIM)),          # 262144 B
    "bv_row": (458752, F32, (1, DIM)),         # 1024 B
    "biasout_row": (459776, F32, (1, DIM)),    # 1024 B
    "sbcol": (460800, F32, (128, 2)),          # 1024 B
    "ab_row": (461824, BF16, (1, DIM)),        # 512 B
}
PK_BYTES = 462336


ZBYTES = N * DIM          # z as fp8e4, 1 B/elem
XBYTES = N * DIM * 2      # x as bf16
ZX_BYTES = ZBYTES + XBYTES


def build_nc(has_ab, has_bias=True):
    nc = bacc.Bacc("TRN2", target_bir_lowering=False, debug=False,
                   enable_asserts=False)
    zxb = nc.dram_tensor("zx", [ZX_BYTES], mybir.dt.uint8, kind="ExternalInput").ap()
    pk = nc.dram_tensor("pk", [PK_BYTES], mybir.dt.uint8, kind="ExternalInput").ap()

    def pk_view(name):
        off, dt, shape = PK_LAYOUT[name]
        esz = mybir.dt.size(dt)
        flat = pk[off : off + esz * shape[0] * shape[1]].bitcast(dt)
        return flat.rearrange("(a b) -> a b", a=shape[0])

    ins = {
        "z": zxb[0:ZBYTES].bitcast(F8).rearrange("(a b) -> a b", a=N),
        "x": zxb[ZBYTES:ZX_BYTES].bitcast(BF16).rearrange("(a b) -> a b", a=N),
        "mq": pk_view("mq"),
        "swv": pk_view("swv"),
        "wv": pk_view("wv"),
        "bv_row": pk_view("bv_row"),
        "biasout_row": pk_view("biasout_row"),
        "sbcol": pk_view("sbcol"),
        "ab_row": pk_view("ab_row") if has_ab else None,
    }
    ins["has_bias"] = has_bias
    outs = {"out": nc.dram_tensor("out", [N, DIM], BF16, kind="ExternalOutput").ap()}
    reps = int(os.environ.get("KREPS", "1"))
    with tile.TileContext(nc) as tc:
        for _ in range(reps):
            build_body(tc, outs, ins)
    nc.compile()
    return nc


LAST_RESULTS = None


def pack_zx(z_core, x_core):
    """Host-side wire packing: z as fp8e4 (the attn path quantizes to fp8
    on-device anyway), x as bf16 (its consumers are bf16/fp8).  Halves+
    HBM load traffic 16 MB -> 6 MB per core."""
    import ml_dtypes

    zb = np.ascontiguousarray(
        z_core.astype(ml_dtypes.float8_e4m3)
    ).view(np.uint8).reshape(-1)
    xb = np.ascontiguousarray(
        x_core.astype(ml_dtypes.bfloat16)
    ).view(np.uint8).reshape(-1)
    return np.concatenate([zb, xb])


def pack_params(p, has_ab):
    """Byte-pack the folded params per PK_LAYOUT into one uint8 tensor."""
    import ml_dtypes

    vals = {
        "mq": p["mq"].astype(ml_dtypes.float8_e4m3),
        "swv": p["swv"].astype(ml_dtypes.bfloat16),
        "wv": p["wv"].astype(np.float32),
        "bv_row": p["bv_row"].astype(np.float32),
        "biasout_row": p["biasout_row"].astype(np.float32),
        "sbcol": p["sbcol"].astype(np.float32),
    }
    if has_ab:
        vals["ab_row"] = p["ab"].reshape(1, DIM).astype(ml_dtypes.bfloat16)
    pk = np.zeros(PK_BYTES, np.uint8)
    for name, arr in vals.items():
        off = PK_LAYOUT[name][0]
        b = np.ascontiguousarray(arr).view(np.uint8).reshape(-1)
        pk[off : off + b.size] = b
    return pk


def kernel(x, z, Wq, bq, K, Wv, bv, alpha, beta):
    global LAST_RESULTS
    import ml_dtypes
    from concourse.bass_utils import run_bass_kernel_spmd

    x = np.ascontiguousarray(np.asarray(x, np.float32))
    z = np.ascontiguousarray(np.asarray(z, np.float32))
    p = fold_params(Wq, bq, K, Wv, bv, alpha, beta)
    has_ab = bool(np.any(p["ab"] != 0.0))
    has_bias = bool(np.any(p["biasout_row"] != 0.0))

    nc = build_nc(has_ab, has_bias)

    pk = pack_params(p, has_ab)
    in_maps = [
        {"zx": pack_zx(z[i], x[i]), "pk": pk} for i in range(NCORES)
    ]
    res = run_bass_kernel_spmd(nc, in_maps, core_ids=list(range(NCORES)))
    LAST_RESULTS = res
    out = np.stack([res.results[i]["out"] for i in range(NCORES)], axis=0)
    return out.astype(np.float32)
